# revision 8
# baseline (speedup 1.0000x reference)
"""DeepMatchingLoss Trainium2 kernel.

Shards the 8 independent (direction, batch) pyramid pipelines across the 8
NeuronCores. Each core computes: correlation matmul -> pyramid level 0
(maxpool + rectify + 2x2 aggregation + border norm) -> level 1 (maxpool +
rectify). The tiny tail (level-1 aggregation, levels 2-3, loss) runs on the
host in float64.

Layout notes (host-prepared):
  - d1 (lhsT, 128 x 784 bf16): atomic-cell axis ordered quadrant-major in
    8 M-tiles of 98 = (quadrant q=(dy,dx), half) x (7 parent rows x 14
    parent cols), so each M-tile's pooled/rectified output is partition-
    aligned with its aggregation target and no partition-gather is needed.
  - d2 (rhs, 128 x 112 x 112 bf16): displacement columns de-interleaved
    ([56 evens | 56 odds] per row) so the column max-pool reads dense
    unit-stride bf16 (DVE 2x mode).
  - cmap0 (128 x 56 x 56 bf16): border-normalization multiplier
    count**(-0.9), replicated across partitions.
"""

import os

os.environ.setdefault("MYCRO_LOCAL_CACHE", "1")

import numpy as np
import ml_dtypes

EPS = 0.03
P_EXP = 1.5
BNORM = 0.9

N_CORES = 8
M_TILE = 98  # 7 parent rows x 14 parent cols per (quadrant, half)

_PROGRAM = None
LAST_RUN = None  # BassKernelResults of the most recent device run (for test.py)


def _build_program(n_tiles=8, n_groups=7, with_l1=True):
    from contextlib import ExitStack

    import concourse.bacc as bacc
    import concourse.mybir as mybir
    import concourse.tile as tile

    BF = mybir.dt.bfloat16
    F32 = mybir.dt.float32
    Relu = mybir.ActivationFunctionType.Relu

    nc = bacc.Bacc("TRN2")
    d1_h = nc.declare_dram_parameter("d1", [128, 784], BF, isOutput=False)
    d2_h = nc.declare_dram_parameter("d2", [128, 112, 112], BF, isOutput=False)
    cm_h = nc.declare_dram_parameter("cmap0", [128, 56, 56], BF, isOutput=False)
    out_h = nc.declare_dram_parameter("r1out", [196, 784], BF, isOutput=True)

    with ExitStack() as ctx:
        tc = ctx.enter_context(tile.TileContext(nc))
        io = ctx.enter_context(tc.tile_pool(name="io", bufs=1))
        persist = ctx.enter_context(tc.tile_pool(name="persist", bufs=1))
        work = ctx.enter_context(tc.tile_pool(name="work", bufs=2))
        psum = ctx.enter_context(tc.tile_pool(name="psum", bufs=2, space="PSUM"))

        d1_s = io.tile([128, 784], BF)
        nc.sync.dma_start(out=d1_s[:], in_=d1_h[:, :])
        d2_s = io.tile([128, 112, 112], BF)
        nc.sync.dma_start(out=d2_s[:], in_=d2_h[:, :, :])
        cm_s = io.tile([128, 56, 56], BF)
        nc.sync.dma_start(out=cm_s[:], in_=cm_h[:, :, :])

        res = []
        for h in range(2):
            rt = persist.tile([98, 56, 56], BF, name=f"res{h}")
            nc.gpsimd.memset(rt[:], 0.0)
            res.append(rt)

        for t in range(n_tiles):
            q, half = divmod(t, 2)
            dy, dx = divmod(q, 2)

            # tmpc: column-pooled (112 disp rows x 56), per M-tile
            tmpc = work.tile([98, 112, 56], BF, tag="tmpc", name=f"tmpc{t}")
            for g in range(n_groups):  # PSUM groups of 16 displacement rows
                pt = psum.tile([98, 4, 512], F32, tag="pt", name=f"pt{t}_{g}")
                for j in range(4):
                    c = 4 * g + j
                    nc.tensor.matmul(
                        pt[:, j, 0:448],
                        d1_s[:, t * 98 : (t + 1) * 98],
                        d2_s[:, 4 * c : 4 * c + 4, :],
                        start=True,
                        stop=True,
                    )
                # Evict PSUM -> SBUF with fused ReLU + fp32->bf16 cast.
                cs = work.tile([98, 16, 112], BF, tag="cs", name=f"cs{t}_{g}")
                nc.scalar.activation(
                    out=cs[:].rearrange("p (a b) c -> p a b c", a=4),
                    in_=pt[:, :, 0:448].rearrange("p a (b c) -> p a b c", c=112),
                    func=Relu,
                )
                # Column pool: tmpc[w'] = max(E[w'], O[w'], O[w'-1])
                rows = slice(16 * g, 16 * g + 16)
                nc.vector.tensor_max(tmpc[:, rows, :], cs[:, :, 0:56], cs[:, :, 56:112])
                nc.vector.tensor_max(
                    tmpc[:, rows, 1:56], tmpc[:, rows, 1:56], cs[:, :, 56:111]
                )
            # Row pool: u[h'] = max(tmpc[2h'], tmpc[2h'+1], tmpc[2h'-1])
            u = work.tile([98, 56, 56], BF, tag="u", name=f"u{t}")
            nc.vector.tensor_max(u[:], tmpc[:, 0:112:2, :], tmpc[:, 1:112:2, :])
            nc.vector.tensor_max(u[:, 1:56, :], u[:, 1:56, :], tmpc[:, 1:110:2, :])
            # Rectify: r = u^1.5 = u * sqrt(u)   (u >= 0 after the ReLU evict)
            s_t = work.tile([98, 56, 56], BF, tag="s", name=f"s{t}")
            nc.scalar.sqrt(s_t[:], u[:])
            nc.vector.tensor_mul(u[:], u[:], s_t[:])
            # Aggregate into parents with the +-1 displacement shift.
            # dy==0 => res rows [1:] += child rows [:-1]; dy==1 the reverse.
            r_out = slice(1, 56) if dy == 0 else slice(0, 55)
            r_in = slice(0, 55) if dy == 0 else slice(1, 56)
            c_out = slice(1, 56) if dx == 0 else slice(0, 55)
            c_in = slice(0, 55) if dx == 0 else slice(1, 56)
            nc.vector.tensor_add(
                res[half][:, r_out, c_out],
                res[half][:, r_out, c_out],
                u[:, r_in, c_in],
            )

        for h in range(2 if with_l1 else 0):
            # Border normalization of level-0 output.
            nc.vector.tensor_mul(res[h][:], res[h][:], cm_s[0:98, :, :])
            # Level-1 column pool (strided reads, natural layout).
            t1 = work.tile([98, 56, 28], BF, tag="t1", name=f"t1_{h}")
            nc.vector.tensor_max(t1[:], res[h][:, :, 0:56:2], res[h][:, :, 1:56:2])
            nc.vector.tensor_max(t1[:, :, 1:28], t1[:, :, 1:28], res[h][:, :, 1:54:2])
            # Level-1 row pool.
            u1 = work.tile([98, 28, 28], BF, tag="u1", name=f"u1_{h}")
            nc.vector.tensor_max(u1[:], t1[:, 0:56:2, :], t1[:, 1:56:2, :])
            nc.vector.tensor_max(u1[:, 1:28, :], u1[:, 1:28, :], t1[:, 1:54:2, :])
            # Level-1 rectify.
            s1 = work.tile([98, 28, 28], BF, tag="s1", name=f"s1_{h}")
            nc.scalar.sqrt(s1[:], u1[:])
            nc.vector.tensor_mul(u1[:], u1[:], s1[:])
            nc.sync.dma_start(
                out=out_h[h * 98 : (h + 1) * 98, :],
                in_=u1[:].rearrange("p a b -> p (a b)"),
            )

    nc.compile()
    return nc


def _get_program():
    global _PROGRAM
    if _PROGRAM is None:
        _PROGRAM = _build_program()
    return _PROGRAM


def _cmap(n):
    rc = np.full(n, 2.0)
    rc[0] = rc[-1] = 1.0
    return np.outer(rc, rc) ** (-BNORM)


def _cell_order():
    """m -> (h, w) atomic-cell indices, quadrant-major in 8 tiles of 98."""
    hs = np.empty(784, np.int64)
    ws = np.empty(784, np.int64)
    m = 0
    for t in range(8):
        q, half = divmod(t, 2)
        dy, dx = divmod(q, 2)
        for il in range(7):
            i = half * 7 + il
            for j in range(14):
                hs[m] = 2 * i + dy
                ws[m] = 2 * j + dx
                m += 1
    return hs, ws


def _prep_inputs(desc1, desc2):
    bf16 = ml_dtypes.bfloat16
    fine = 16 + 2 * np.arange(112)
    coarse = 20 + 8 * np.arange(28)
    colperm = np.concatenate([np.arange(0, 112, 2), np.arange(1, 112, 2)])
    hs, ws = _cell_order()
    cm0 = np.ascontiguousarray(
        np.broadcast_to(_cmap(56).astype(bf16), (128, 56, 56))
    )

    in_maps = []
    for dirn in range(2):
        A = desc1 if dirn == 0 else desc2  # coarse (query) descriptor
        B = desc2 if dirn == 0 else desc1  # fine (target) descriptor
        for b in range(4):
            d1g = A[b][:, coarse][:, :, coarse]  # (128, 28, 28)
            d1p = np.ascontiguousarray(d1g[:, hs, ws].astype(bf16))  # (128, 784)
            d2g = B[b][:, fine][:, :, fine]  # (128, 112, 112)
            d2p = np.ascontiguousarray(d2g[:, :, colperm].astype(bf16))
            in_maps.append({"d1": d1p, "d2": d2p, "cmap0": cm0})
    return in_maps


def _sl(v):
    return slice(0, -1) if v < 0 else slice(1, None)


def _pool3s2(x):
    """3x3 stride-2 pad-1 max pool over the last two dims (even sizes)."""
    H, W = x.shape[-2:]
    Ho, Wo = H // 2, W // 2
    xp = np.full(x.shape[:-2] + (H + 2, W + 2), -np.inf, x.dtype)
    xp[..., 1:-1, 1:-1] = x
    out = np.full(x.shape[:-2] + (Ho, Wo), -np.inf, x.dtype)
    for di in range(3):
        for dj in range(3):
            np.maximum(
                out,
                xp[..., di : di + 2 * Ho : 2, dj : dj + 2 * Wo : 2],
                out=out,
            )
    return out


def _agg(rect):
    """2x2 children -> parents with +-1 displacement shifts + border norm."""
    H1 = rect.shape[0]
    Hp = H1 // 2
    D = rect.shape[-1]
    res = np.zeros((Hp, Hp, D, D), rect.dtype)
    for y in (-1, 1):
        for x in (-1, 1):
            dy, dx = (y + 1) // 2, (x + 1) // 2
            child = rect[dy : dy + 2 * Hp : 2, dx : dx + 2 * Hp : 2]
            res[:, :, _sl(-y), _sl(-x)] += child[:, :, _sl(y), _sl(x)]
    return res * _cmap(D)


def _host_tail(r1_cores):
    """r1_cores: (8, 196, 784) float64 level-1 rectified outputs."""
    # device row -> (i, j) cell of the 14x14 level-1 grid
    ii = np.arange(14)
    idx = (ii[:, None] // 7) * 98 + (ii[:, None] % 7) * 14 + np.arange(14)[None, :]
    total = 0.0
    for k in range(8):
        rect1 = r1_cores[k].reshape(196, 28, 28)[idx]  # (14, 14, 28, 28)
        res1 = _agg(rect1)  # (7, 7, 28, 28)
        rect2 = np.maximum(_pool3s2(res1), 0.0) ** P_EXP
        res2 = _agg(rect2)  # (3, 3, 14, 14)
        rect3 = np.maximum(_pool3s2(res2), 0.0) ** P_EXP
        res3 = _agg(rect3)  # (1, 1, 7, 7)
        r = np.maximum(res3, 0.0) ** P_EXP
        denom = r.sum() + EPS
        total += ((r / denom) ** 2).sum()
    return np.array(-total / 8.0, dtype=np.float32)


def kernel(desc1, desc2):
    global LAST_RUN
    from concourse.bass_utils import run_bass_kernel_spmd

    nc = _get_program()
    in_maps = _prep_inputs(np.asarray(desc1), np.asarray(desc2))
    LAST_RUN = run_bass_kernel_spmd(nc, in_maps, list(range(N_CORES)))
    r1 = np.stack(
        [LAST_RUN.results[k]["r1out"].astype(np.float64) for k in range(N_CORES)]
    )
    return _host_tail(r1)
